# revision 113
# baseline (speedup 1.0000x reference)
"""MambaVisionBlock Trainium2 Bass kernel (v3).

Sharding: data-parallel over batch B=8 across 8 NeuronCores (1 batch/core),
all parameters replicated.  Per-core problem: x [4096, 256].

Design (baseline v1 was 380us):
 - Token-major LayerNorm: per-128-token-block stats via accum_out, ONE
   batched finalize per LN (single Sqrt instruction keeps the Act function
   table stable: 4 table loads total), fused (x*rstd + alpha) normalize as
   tensor_scalar ops in the f32-SBUF 2x mode.
 - x loaded + LN1 stats emitted BEFORE weight setup so the DVE/Pool queues
   start immediately; weight transposes (PE) overlap the stats.
 - fp8(e4m3) DoubleRow matmuls for in_proj / fc1 / fc2 (2 k-tiles per
   instruction at 0.5 cyc/row = 4x bf16 throughput); conv and out_proj stay
   bf16; residual stream f32.
 - Depthwise conv(k=3) on the PE as 3 shifted diagonal-lhsT matmuls.
 - LN affine folded into in_proj/fc1 weights; ssm bc/D folded into two
   scaled copies of out_proj^T summed in PSUM.
 - cumsum chain split across DVE (half) and Pool (half).
"""

import sys

if "/opt/trn_rl_repo" not in sys.path:
    sys.path.insert(0, "/opt/trn_rl_repo")

import numpy as np

B, L, D = 8, 4096, 256
Dff = 1024
T = 512            # token chunk
NCH = L // T       # 8 chunks
NCORES = 8
LN_EPS = 1e-5

_CACHE = {}

WEIGHT_NAMES = [
    "ln1_w", "ln1_b", "in_proj_w", "conv_w", "conv_b", "ssm_B", "ssm_C",
    "ssm_D", "out_proj_w", "ln2_w", "ln2_b", "fc1_w", "fc1_b", "fc2_w",
    "fc2_b",
]


def _build():
    import concourse.tile as tile
    from concourse import bacc, mybir
    from concourse.masks import make_identity

    f32 = mybir.dt.float32
    bf16 = mybir.dt.bfloat16
    f8 = mybir.dt.float8e4
    ALU = mybir.AluOpType
    ACT = mybir.ActivationFunctionType
    DR = mybir.MatmulPerfMode.DoubleRow

    nc = bacc.Bacc(trn_type="TRN2")

    # ---- DRAM I/O ----
    x_h = nc.dram_tensor("x", [L, D], f32, kind="ExternalInput")
    w_h = {}
    shapes = {
        "ln1_w": [D], "ln1_b": [D], "in_proj_w": [2 * D, D],
        "conv_w": [D, 1, 3], "conv_b": [D], "ssm_B": [D, 8], "ssm_C": [D, 8],
        "ssm_D": [D], "out_proj_w": [D, D], "ln2_w": [D], "ln2_b": [D],
        "fc1_w": [Dff, D], "fc1_b": [Dff], "fc2_w": [D, Dff], "fc2_b": [D],
    }
    for n in WEIGHT_NAMES:
        w_h[n] = nc.dram_tensor(n, shapes[n], f32, kind="ExternalInput")
    out_h = nc.dram_tensor("out", [L, D], f32, kind="ExternalOutput")

    from contextlib import ExitStack
    with tile.TileContext(nc) as tc, ExitStack() as stack:
        pw = stack.enter_context(tc.tile_pool(name="weights", bufs=1))
        pbig = stack.enter_context(tc.tile_pool(name="big", bufs=1))
        pa = stack.enter_context(tc.tile_pool(name="acts", bufs=2))
        pa3 = stack.enter_context(tc.tile_pool(name="acts3", bufs=3))
        ps_tr = stack.enter_context(tc.tile_pool(name="ps_tr", bufs=2, space="PSUM"))
        ps_xz = stack.enter_context(tc.tile_pool(name="ps_xz", bufs=1, space="PSUM"))
        ps_cv = stack.enter_context(tc.tile_pool(name="ps_cv", bufs=1, space="PSUM"))
        ps_f = stack.enter_context(tc.tile_pool(name="ps_f", bufs=1, space="PSUM"))

        # ---- constants ----
        i32 = mybir.dt.int32
        magic_i = pw.tile([128, 32], i32, tag="magic_i")
        nc.vector.memset(magic_i, 0x5F3759DF)
        one_i = pw.tile([128, 1], i32, tag="one_i")
        nc.vector.memset(one_i, 1)
        ident = pw.tile([128, 128], f32, tag="ident")
        make_identity(nc, ident)
        ident_bf = pw.tile([128, 128], bf16, tag="ident_bf")
        nc.vector.tensor_copy(ident_bf, ident)
        zeros_bf = pw.tile([128, T], bf16, tag="zeros_bf")
        nc.vector.memset(zeros_bf, 0.0)
        epsv = pw.tile([128, 1], f32, tag="eps")
        nc.vector.memset(epsv, LN_EPS)

        # ---- big activations ----
        x_tm = pbig.tile([128, NCH, 4, 256], f32, tag="x_tm")
        r1_tm = pbig.tile([128, NCH, 4, 256], f32, tag="r1_tm")
        h1 = pbig.tile([128, 2, L], f8, tag="h1")
        h2 = pbig.tile([128, 2, L], f8, tag="h2")

        # ---- token-major LN helpers ----
        def ln_stats(src_big, c, s1v, s2v, tag):
            # DVE path: accum_out reductions
            src = src_big[:, c]
            scr = pa.tile([128, 256], bf16, tag="scr" + tag)
            for s in range(4):
                nc.vector.tensor_scalar(out=scr, in0=src[:, s], scalar1=1.0,
                                        scalar2=0.0, op0=ALU.mult, op1=ALU.add,
                                        accum_out=s1v[:, c * 4 + s:c * 4 + s + 1])
            scrd = pa.tile([128, 256], bf16, tag="scrd" + tag)
            for s in range(4):
                nc.vector.scalar_tensor_tensor(
                    out=scrd, in0=src[:, s], scalar=1.0, in1=src[:, s],
                    op0=ALU.mult, op1=ALU.mult,
                    accum_out=s2v[:, c * 4 + s:c * 4 + s + 1])

        def ln_stats_mixed(src_big, c, s1v, s2v, tag, act_k, skip_s1=False):
            # s1 on DVE; s2 split: the first act_k s-blocks go through the
            # Act engine (Square + accum_out), the rest stay on DVE.
            src = src_big[:, c]
            if not skip_s1:
                scr = pa.tile([128, 256], bf16, tag="scm" + tag)
                for s in range(4):
                    nc.vector.tensor_scalar(
                        out=scr, in0=src[:, s], scalar1=1.0,
                        scalar2=0.0, op0=ALU.mult, op1=ALU.add,
                        accum_out=s1v[:, c * 4 + s:c * 4 + s + 1])
            scra = pa.tile([128, 256], bf16, tag="sca" + tag)
            scrd = pa.tile([128, 256], bf16, tag="scd" + tag)
            for s in range(4):
                j = c * 4 + s
                if s < act_k:
                    nc.scalar.activation(scra, src[:, s], ACT.Square,
                                         accum_out=s2v[:, j:j + 1])
                else:
                    nc.vector.scalar_tensor_tensor(
                        out=scrd, in0=src[:, s], scalar=1.0, in1=src[:, s],
                        op0=ALU.mult, op1=ALU.mult,
                        accum_out=s2v[:, j:j + 1])

        def ln_tiles(tag):
            rstd = pw.tile([128, 32], f32, tag="rstd" + tag)
            alpha = pw.tile([128, 32], f32, tag="al" + tag)
            mun = pw.tile([128, 32], f32, tag="mun" + tag)
            var = pw.tile([128, 32], f32, tag="var" + tag)
            sd = pw.tile([128, 32], f32, tag="sd" + tag)
            return (rstd, alpha, mun, var, sd)

        def ln_finalize(s1v, s2v, tiles, lo, hi, tag=""):
            # finalize columns [lo:hi) only — lets the first half of the
            # chunks proceed before the second half's stats are in
            rstd, alpha, mun, var, sd = tiles
            cs = slice(lo, hi)
            nc.vector.tensor_scalar_mul(mun[:, cs], s1v[:, cs], -1.0 / D)
            nc.vector.tensor_mul(var[:, cs], mun[:, cs], mun[:, cs])
            nc.vector.scalar_tensor_tensor(
                out=var[:, cs], in0=s2v[:, cs], scalar=1.0 / D, in1=var[:, cs],
                op0=ALU.mult, op1=ALU.subtract)
            # rstd = rsqrt(var + eps) fully on DVE: fast-inverse-sqrt seed
            # (0x5f3759df bithack) + two Newton steps -> ~1e-6 relative, no
            # Act Sqrt (keeps the activation table stable all kernel long)
            nc.vector.tensor_scalar_add(sd[:, cs], var[:, cs], LN_EPS)
            sh = pw.tile([128, 32], i32, tag="sh" + tag)
            nc.vector.tensor_scalar(out=sh[:, cs],
                                    in0=sd[:, cs].bitcast(i32),
                                    scalar1=one_i[:, 0:1], scalar2=None,
                                    op0=ALU.arith_shift_right)
            y = pw.tile([128, 32], f32, tag="y" + tag)
            nc.vector.tensor_tensor(out=y[:, cs].bitcast(i32),
                                    in0=magic_i[:, cs], in1=sh[:, cs],
                                    op=ALU.subtract)
            t_ = pw.tile([128, 32], f32, tag="t" + tag)
            for _ in range(2):
                nc.vector.tensor_mul(t_[:, cs], y[:, cs], y[:, cs])
                nc.vector.tensor_mul(t_[:, cs], t_[:, cs], sd[:, cs])
                nc.vector.tensor_scalar(out=t_[:, cs], in0=t_[:, cs],
                                        scalar1=-0.5, scalar2=1.5,
                                        op0=ALU.mult, op1=ALU.add)
                nc.vector.tensor_mul(y[:, cs], y[:, cs], t_[:, cs])
            nc.vector.tensor_copy(rstd[:, cs], y[:, cs])
            nc.vector.tensor_mul(alpha[:, cs], mun[:, cs], rstd[:, cs])
            return rstd, alpha

        def ln_norm(src_big, dst_big, c, rstd, alpha, tag, copy_eng=None,
                    norm_eng=None):
            src = src_big[:, c]
            hn = pa.tile([128, 4, 256], bf16, tag="hn" + tag)
            for s in range(4):
                j = c * 4 + s
                if norm_eng == "act":
                    nc.scalar.activation(hn[:, s], src[:, s], ACT.Identity,
                                         scale=rstd[:, j:j + 1],
                                         bias=alpha[:, j:j + 1])
                else:
                    nc.vector.tensor_scalar(
                        out=hn[:, s], in0=src[:, s], scalar1=rstd[:, j:j + 1],
                        scalar2=alpha[:, j:j + 1], op0=ALU.mult, op1=ALU.add)
            for db in range(2):
                ps = ps_tr.tile([128, 2, 256], bf16, tag="trb")
                for s in range(4):
                    nc.tensor.transpose(
                        ps[:, s // 2, (s % 2) * 128:(s % 2) * 128 + 128],
                        hn[:, s, db * 128:(db + 1) * 128], ident_bf)
                if copy_eng == "dve":
                    nc.vector.tensor_copy(dst_big[:, db, c * T:(c + 1) * T],
                                          ps.rearrange("p a b -> p (a b)"))
                else:
                    nc.scalar.copy(dst_big[:, db, c * T:(c + 1) * T],
                                   ps.rearrange("p a b -> p (a b)"))

        # ================= P1a: load x + LN1 stats =================
        # DMA issue order tuned for the serialized DMA device: x chunks 0-3
        # first (they gate the first LN finalize half), small LN vectors and
        # in/out_proj staging next, then x 4-7 and the fc staging.
        s1v1 = pw.tile([128, 32], f32, tag="s1v1")
        s2v1 = pw.tile([128, 32], f32, tag="s2v1")

        def vec_tile(name, nblk):
            t_ = pw.tile([128, nblk], f32, tag="v_" + name)
            nc.sync.dma_start(out=t_, in_=w_h[name][:].rearrange("(b p) -> p b", p=128))
            return t_

        def x_load(g_, eng):
            tok = slice(g_ * 2 * T, (g_ + 1) * 2 * T)
            eng.dma_start(out=x_tm[:, 2 * g_:2 * g_ + 2],
                          in_=x_h[tok, :].rearrange("(s p) d -> p s d", p=128))

        x_load(0, nc.sync)
        x_load(1, nc.scalar)
        ln1w = vec_tile("ln1_w", 2)
        ln1b = vec_tile("ln1_b", 2)
        ln2w = vec_tile("ln2_w", 2)
        ln2b = vec_tile("ln2_b", 2)
        st_in = pw.tile([128, 4, 256], f32, tag="wst_a")
        nc.sync.dma_start(out=st_in, in_=w_h["in_proj_w"][:].rearrange("(e p) d -> p e d", p=128))
        st_out = pw.tile([128, 2, 256], f32, tag="wst_b")
        nc.sync.dma_start(out=st_out, in_=w_h["out_proj_w"][:].rearrange("(e p) d -> p e d", p=128))
        cw = pw.tile([128, 2, 3], f32, tag="convw")
        nc.sync.dma_start(out=cw, in_=w_h["conv_w"][:, 0, :].rearrange("(b p) k -> p b k", p=128))
        convb = vec_tile("conv_b", 2)
        ssmD = vec_tile("ssm_D", 2)
        ssmB = pw.tile([128, 2, 8], f32, tag="ssmB")
        nc.sync.dma_start(out=ssmB, in_=w_h["ssm_B"][:].rearrange("(b p) s -> p b s", p=128))
        ssmC = pw.tile([128, 2, 8], f32, tag="ssmC")
        nc.sync.dma_start(out=ssmC, in_=w_h["ssm_C"][:].rearrange("(b p) s -> p b s", p=128))
        for c in range(4):
            ln_stats_mixed(x_tm, c, s1v1, s2v1, "1", act_k=4)
        x_load(2, nc.sync)
        x_load(3, nc.scalar)
        # fc staging comes after the x loads: it is only needed for P4 / the
        # LN2-side bias folds.
        st_f1 = pw.tile([128, 8, 256], f32, tag="wst_c")
        nc.scalar.dma_start(out=st_f1, in_=w_h["fc1_w"][:].rearrange("(e p) d -> p e d", p=128))
        st_f2 = pw.tile([128, 2, 1024], f32, tag="wst_d")
        nc.scalar.dma_start(out=st_f2, in_=w_h["fc2_w"][:].rearrange("(e p) f -> p e f", p=128))
        fc1b = vec_tile("fc1_b", 8)
        fc2b = vec_tile("fc2_b", 2)

        # ================= weight setup (overlaps the stats) =================
        bcprod = pw.tile([128, 2, 8], f32, tag="bcprod")
        nc.vector.tensor_mul(bcprod, ssmB, ssmC)
        bc = pw.tile([128, 2], f32, tag="bc")
        nc.vector.tensor_reduce(bc, bcprod, axis=mybir.AxisListType.X, op=ALU.add)

        def ts_copy(dst, src_ps, scale):
            if scale is None:
                nc.vector.tensor_copy(dst, src_ps)
            else:
                nc.vector.tensor_scalar_mul(dst, src_ps, scale)

        # in_proj: bf16 copy (for bias matvec) + fp8 copy (DoubleRow lhsT),
        # rows scaled by ln1_w (LN1 affine fold).
        w_inT = pw.tile([128, 2, 512], bf16, tag="w_inT")
        w_inF = pw.tile([128, 2, 512], f8, tag="w_inF")
        ps = ps_xz.tile([128, 2, T], f32, tag="xz")
        for db in range(2):
            for eb in range(4):
                nc.tensor.transpose(ps[:, db, eb * 128:(eb + 1) * 128],
                                    st_in[:, eb, db * 128:(db + 1) * 128], ident)
            ts_copy(w_inT[:, db], ps[:, db], ln1w[:, db:db + 1])
            nc.vector.tensor_copy(w_inF[:, db], w_inT[:, db])

        # out_proj: two fp8 scaled lhsT copies (bc / ssm_D folds).
        w_bcT = pw.tile([128, 2, 256], f8, tag="w_bcT")
        w_DT = pw.tile([128, 2, 256], f8, tag="w_DT")
        ps = ps_xz.tile([128, 2, T], f32, tag="xz")
        for db in range(2):
            for ob in range(2):
                nc.tensor.transpose(ps[:, db, ob * 128:(ob + 1) * 128],
                                    st_out[:, ob, db * 128:(db + 1) * 128], ident)
            ts_copy(w_bcT[:, db], ps[:, db, 0:256], bc[:, db:db + 1])
            ts_copy(w_DT[:, db], ps[:, db, 0:256], ssmD[:, db:db + 1])

        # conv diag lhsT: diag(w_k) per (k, d-block), bf16
        diagw = []
        for k in range(3):
            row = []
            for b_ in range(2):
                d_ = pw.tile([128, 128], bf16, tag=f"diag{k}{b_}")
                nc.vector.tensor_scalar_mul(d_, ident, cw[:, b_, k:k + 1])
                row.append(d_)
            diagw.append(row)

        # ---- LN1-side bias folds (exact when ln1_b == 0) ----
        ln1b_bf = pw.tile([128, 2], bf16, tag="ln1b_bf")
        nc.vector.tensor_copy(ln1b_bf, ln1b)

        ps_bt = ps_xz.tile([128, 2, T], f32, tag="xz")
        ps_b = ps_bt[:, 0]
        for j in range(4):
            for db in range(2):
                nc.tensor.matmul(ps_b[:, j:j + 1],
                                 w_inT[:, db, j * 128:(j + 1) * 128],
                                 ln1b_bf[:, db:db + 1],
                                 start=(db == 0), stop=(db == 1))
        beta_xc = pw.tile([128, 2], f32, tag="beta_xc")
        nc.vector.tensor_copy(beta_xc, ps_b[:, 0:2])
        bias_z = pw.tile([128, 2], f32, tag="bias_z")
        nc.vector.tensor_copy(bias_z, ps_b[:, 2:4])

        # conv silu bias: conv_b + beta_xc*(w0+w1+w2); chunk-0 edge corr.
        wsum = pw.tile([128, 2], f32, tag="wsum")
        nc.vector.tensor_reduce(wsum, cw, axis=mybir.AxisListType.X, op=ALU.add)
        bias_xc = pw.tile([128, 2], f32, tag="bias_xc")
        nc.vector.scalar_tensor_tensor(out=bias_xc, in0=beta_xc, scalar=1.0,
                                       in1=wsum, op0=ALU.mult, op1=ALU.mult)
        nc.vector.tensor_add(bias_xc, bias_xc, convb)
        w01 = pw.tile([128, 2], f32, tag="w01")
        nc.vector.tensor_add(w01, cw[:, :, 0], cw[:, :, 1])
        corr = pw.tile([128, 2, 2], f32, tag="corr")
        nc.vector.tensor_mul(corr[:, :, 0], beta_xc, w01)
        nc.vector.tensor_mul(corr[:, :, 1], beta_xc, cw[:, :, 0])
        nc.vector.tensor_scalar_mul(corr, corr, -1.0)

        # Bias rows on partition 0 for the K=1 bias matmuls, extracted via
        # identity matmuls (row_j = col_j^T @ I): biasZC = [z0 z1 cv0 cv1].
        ones_row = pw.tile([1, T], bf16, tag="ones_row")
        nc.vector.memset(ones_row, 1.0)
        zc_bf = pw.tile([128, 4], bf16, tag="zc_bf")
        nc.vector.tensor_copy(zc_bf[:, 0:2], bias_z)
        nc.vector.tensor_copy(zc_bf[:, 2:4], bias_xc)
        ps_rowt = ps_xz.tile([128, 2, T], f32, tag="xz")
        ps_row = ps_rowt[0:1, 0]
        for j in range(4):
            nc.tensor.matmul(ps_row[:, j * 128:(j + 1) * 128],
                             zc_bf[:, j:j + 1], ident_bf)
        biasZC = pw.tile([1, 512], bf16, tag="biasZC")
        nc.vector.tensor_copy(biasZC, ps_row)

        # ================= P1b: LN1 finalize (1st half) =================
        t1 = ln_tiles("1")
        rstd1, alpha1 = ln_finalize(s1v1, s2v1, t1, 0, 16, tag="1")
        for c in range(4):
            ln_norm(x_tm, h1, c, rstd1, alpha1, "1")

        # ---- fc1/fc2 setup (needed from P4; overlaps P1/P2) ----
        w1T = pw.tile([128, 2, 1024], bf16, tag="w1T")
        w1F = pw.tile([128, 2, 1024], f8, tag="w1F")
        for db in range(2):
            ps = ps_xz.tile([128, 2, T], f32, tag="xz")
            for half in range(2):
                for i in range(4):
                    fb = half * 4 + i
                    nc.tensor.transpose(ps[:, half, i * 128:(i + 1) * 128],
                                        st_f1[:, fb, db * 128:(db + 1) * 128], ident)
                ts_copy(w1T[:, db, half * 512:(half + 1) * 512], ps[:, half],
                        ln2w[:, db:db + 1])
            nc.vector.tensor_copy(w1F[:, db], w1T[:, db])

        w2F = pw.tile([128, 8, 256], f8, tag="w2F")
        for ob in range(2):
            ps = ps_xz.tile([128, 2, T], f32, tag="xz")
            for half in range(2):
                for i in range(4):
                    fb = half * 4 + i
                    nc.tensor.transpose(ps[:, half, i * 128:(i + 1) * 128],
                                        st_f2[:, ob, fb * 128:(fb + 1) * 128], ident)
                nc.vector.tensor_copy(
                    w2F[:, half * 4:(half + 1) * 4, ob * 128:(ob + 1) * 128],
                    ps[:, half].rearrange("p (a b) -> p a b", a=4))

        ln2b_bf = pw.tile([128, 2], bf16, tag="ln2b_bf")
        nc.vector.tensor_copy(ln2b_bf, ln2b)
        ps_b2t = ps_xz.tile([128, 2, T], f32, tag="xz")
        ps_b2 = ps_b2t[:, 0]
        for fb in range(8):
            for db in range(2):
                nc.tensor.matmul(ps_b2[:, fb:fb + 1],
                                 w1T[:, db, fb * 128:(fb + 1) * 128],
                                 ln2b_bf[:, db:db + 1],
                                 start=(db == 0), stop=(db == 1))
        bias1 = pw.tile([128, 8], f32, tag="bias1")
        nc.vector.tensor_add(bias1, ps_b2[:, 0:8], fc1b)
        b1_bf = pw.tile([128, 8], bf16, tag="b1_bf")
        nc.vector.tensor_copy(b1_bf, bias1)
        ps_row2t = ps_xz.tile([128, 2, T], f32, tag="xz")
        ps_row2 = ps_row2t[0:1]
        for j in range(8):
            nc.tensor.matmul(ps_row2[:, j // 4, (j % 4) * 128:(j % 4) * 128 + 128],
                             b1_bf[:, j:j + 1], ident_bf)
        biasF = pw.tile([1, 1024], bf16, tag="biasF")
        nc.vector.tensor_copy(biasF,
                              ps_row2.rearrange("p a b -> p (a b)"))

        # (LN1's 2nd half is interleaved into the P2 loop below so the DVE
        # queue alternates between P1 tail work and P2 back-end work.)

        # ================= P2: mixer (software-pipelined) =================
        # front(c): in_proj + conv + silus; back(c): scan + y*z + out_proj +
        # transpose + residual + LN2 stats.  back(c-1) is emitted after
        # front(c) so chunk c's front is never queued behind chunk c-1's
        # back-end on the in-order engines.
        s1v2 = pw.tile([128, 32], f32, tag="s1v2")
        s2v2 = pw.tile([128, 32], f32, tag="s2v2")
        t2 = ln_tiles("2")
        state = {}

        def p2_front(c):
            hs = h1[:, :, c * T:(c + 1) * T]
            pz = ps_xz.tile([128, 2, T], f32, tag="xz")
            for zb in range(2):
                nc.tensor.matmul(pz[:, zb],
                                 biasZC[:, zb * 128:(zb + 1) * 128],
                                 ones_row, start=True, stop=False)
                nc.tensor.matmul(pz[:, zb],
                                 w_inF[:, :, (2 + zb) * 128:(3 + zb) * 128],
                                 hs, start=False, stop=True, perf_mode=DR)
            zt = pa3.tile([128, 2, T], bf16, tag="zt")
            nc.scalar.activation(zt, pz, ACT.Silu)
            px = ps_xz.tile([128, 2, T], f32, tag="xz")
            for eb in range(2):
                nc.tensor.matmul(px[:, eb],
                                 w_inF[:, :, eb * 128:(eb + 1) * 128],
                                 hs, start=True, stop=True, perf_mode=DR)
            xc = pa3.tile([128, 2, T + 2], bf16, tag="xc")
            nc.scalar.copy(xc[:, :, 2:], px)
            if c == 0:
                nc.vector.memset(xc[:, :, 0:2], 0.0)
            else:
                nc.vector.tensor_copy(xc[:, :, 0:2],
                                      state[c - 1]["xc"][:, :, T:T + 2])
            pc = ps_cv.tile([128, 2, T], f32, tag="cv")
            for eb in range(2):
                nc.tensor.matmul(pc[:, eb],
                                 biasZC[:, (2 + eb) * 128:(3 + eb) * 128],
                                 ones_row, start=True, stop=False)
                for k in range(3):
                    nc.tensor.matmul(pc[:, eb], diagw[k][eb],
                                     xc[:, eb, k:k + T],
                                     start=False, stop=(k == 2))
            if c == 0:
                nc.vector.tensor_add(pc[:, :, 0:2], pc[:, :, 0:2], corr)
            xcv = pa3.tile([128, 2, T], bf16, tag="xcv")
            nc.scalar.activation(xcv, pc, ACT.Silu)
            state[c] = {"zt": zt, "xc": xc, "xcv": xcv}

        def p2_back(c):
            st = state[c]
            zt, xcv = st["zt"], st["xcv"]
            cum = pa3.tile([128, 2, T], bf16, tag="cum")
            for eb in range(2):
                init = 0.0 if c == 0 else state[c - 1]["cum"][:, eb, T - 1:T]
                nc.vector.tensor_tensor_scan(
                    out=cum[:, eb], data0=xcv[:, eb], data1=zeros_bf,
                    initial=init, op0=ALU.add, op1=ALU.add)
            st["cum"] = cum
            cumz = pa3.tile([128, 2, T], f8, tag="cumz")
            nc.vector.tensor_mul(cumz, cum, zt)
            xcvz = pa3.tile([128, 2, T], f8, tag="xcvz")
            nc.gpsimd.tensor_mul(xcvz, xcv, zt)
            po = ps_f.tile([128, 2, T], f32, tag="f")
            for ob in range(2):
                obs = slice(ob * 128, (ob + 1) * 128)
                nc.tensor.matmul(po[:, ob], w_bcT[:, :, obs], cumz,
                                 start=True, stop=False, perf_mode=DR)
                nc.tensor.matmul(po[:, ob], w_DT[:, :, obs], xcvz,
                                 start=False, stop=True, perf_mode=DR)
            op_sb = pa3.tile([128, 2, T], bf16, tag="op_sb")
            nc.scalar.copy(op_sb, po)
            for pair in range(2):
                ps = ps_tr.tile([128, 2, 256], bf16, tag="trb")
                for si in range(2):
                    s = pair * 2 + si
                    for ob in range(2):
                        nc.tensor.transpose(
                            ps[:, si, ob * 128:(ob + 1) * 128],
                            op_sb[:, ob, s * 128:(s + 1) * 128], ident_bf)
                # fused: r1 = x + mix AND accumulate LN2's per-block sum
                for si in range(2):
                    s = pair * 2 + si
                    j = c * 4 + s
                    nc.vector.scalar_tensor_tensor(
                        out=r1_tm[:, c, s], in0=x_tm[:, c, s], scalar=1.0,
                        in1=ps[:, si], op0=ALU.mult, op1=ALU.add,
                        accum_out=s1v2[:, j:j + 1])
            ln_stats_mixed(r1_tm, c, s1v2, s2v2, "2", act_k=0, skip_s1=True)
            if c > 1:
                del state[c - 2]

        for c in range(NCH):
            p2_front(c)
            if c > 0:
                p2_back(c - 1)
            # LN1 2nd half, spread across the first P2 iterations
            if c < 2:
                cc0 = c * 2 + 4
                ln_stats_mixed(x_tm, cc0, s1v1, s2v1, "1", act_k=4)
                ln_stats_mixed(x_tm, cc0 + 1, s1v1, s2v1, "1", act_k=4)
            if c == 1:
                ln_finalize(s1v1, s2v1, t1, 16, 32, tag="1")
            if 1 <= c < 5:
                ln_norm(x_tm, h1, c + 3, rstd1, alpha1, "1")
            # finalize LN2's first half as soon as r1(0..3) exists so the
            # first h2 chunks are ready before P2 fully drains
            if c == 4:
                rstd2, alpha2 = ln_finalize(s1v2, s2v2, t2, 0, 16, tag="2")
                ln_norm(r1_tm, h2, 0, rstd2, alpha2, "2")
                ln_norm(r1_tm, h2, 1, rstd2, alpha2, "2")
            if c == 5:
                ln_norm(r1_tm, h2, 2, rstd2, alpha2, "2")
                ln_norm(r1_tm, h2, 3, rstd2, alpha2, "2")
        # ================= P4: MLP (fp8 DoubleRow) =================
        def p4_chunk(c):
            tok = slice(c * T, (c + 1) * T)
            hs = h2[:, :, tok]
            pf2 = ps_xz.tile([128, 2, T], f32, tag="xz")
            for q in range(4):
                pf = (ps_f if q % 2 == 0 else ps_cv).tile(
                    [128, 2, T], f32, tag=("f" if q % 2 == 0 else "cv"))
                g = pa.tile([128, 2, T], f8, tag="g")
                for i in range(2):
                    fb = q * 2 + i
                    nc.tensor.matmul(pf[:, i],
                                     biasF[:, fb * 128:(fb + 1) * 128],
                                     ones_row, start=True, stop=False)
                    nc.tensor.matmul(pf[:, i],
                                     w1F[:, :, fb * 128:(fb + 1) * 128],
                                     hs, start=False, stop=True, perf_mode=DR)
                nc.scalar.activation(g, pf, ACT.Gelu)
                for ob in range(2):
                    obs = slice(ob * 128, (ob + 1) * 128)
                    nc.tensor.matmul(pf2[:, ob], w2F[:, 2 * q:2 * q + 2, obs],
                                     g, start=(q == 0), stop=(q == 3),
                                     perf_mode=DR)
            ofm = pa.tile([128, 2, T], bf16, tag="ofm")
            for ob in range(2):
                nc.vector.tensor_scalar_add(ofm[:, ob], pf2[:, ob],
                                            fc2b[:, ob:ob + 1])
            out_tm = pa.tile([128, 4, 256], f32, tag="out_tm")
            for pair in range(2):
                ps = ps_tr.tile([128, 2, 256], bf16, tag="trb")
                for si in range(2):
                    s = pair * 2 + si
                    for ob in range(2):
                        nc.tensor.transpose(
                            ps[:, si, ob * 128:(ob + 1) * 128],
                            ofm[:, ob, s * 128:(s + 1) * 128], ident_bf)
                nc.vector.tensor_add(out_tm[:, pair * 2:(pair + 1) * 2],
                                     r1_tm[:, c, pair * 2:(pair + 1) * 2], ps)
            nc.sync.dma_start(out=out_h[tok, :].rearrange("(s p) d -> p s d", p=128),
                              in_=out_tm)

        # first MLP chunks interleave with the P2 tail (their h2 is ready)
        p4_chunk(0)
        p2_back(NCH - 1)
        p4_chunk(1)

        # ================= P3: LN2 finalize (2nd half) =================
        rstd2, alpha2 = ln_finalize(s1v2, s2v2, t2, 16, 32, tag="2")
        for c in range(4, NCH):
            ln_norm(r1_tm, h2, c, rstd2, alpha2, "2")
        for c in range(2, NCH):
            p4_chunk(c)

    nc.compile()
    return nc


def _get_nc():
    if "nc" not in _CACHE:
        _CACHE["nc"] = _build()
    return _CACHE["nc"]


_LAST_RESULTS = None


def kernel(**inputs) -> np.ndarray:
    global _LAST_RESULTS
    from concourse.bass_utils import run_bass_kernel_spmd

    nc = _get_nc()
    x = np.asarray(inputs["x"], np.float32)
    weights = {n: np.ascontiguousarray(np.asarray(inputs[n], np.float32))
               for n in WEIGHT_NAMES}
    in_maps = []
    for core in range(NCORES):
        m = {"x": np.ascontiguousarray(x[core])}
        m.update(weights)
        in_maps.append(m)
    res = run_bass_kernel_spmd(nc, in_maps, core_ids=list(range(NCORES)))
    _LAST_RESULTS = res
    return np.stack([r["out"] for r in res.results], axis=0)


if __name__ == "__main__":
    print("smoke build only")
    _get_nc()
    print("build OK")
